# revision 6
# baseline (speedup 1.0000x reference)
"""Bass/Trainium2 kernel for nn_PhysicsLoss (GNN message passing physics loss).

Scan-based segment-sum design, 8-way edge sharding (400K edges/core):

Host (numpy): sorts each core's edges by dst (order-1) and by src
(order-2), precomputes per-node run-boundary positions (searchsorted)
and the permutation pi mapping order-2 slots to order-1 positions.

Device per core:
  1. per-partition indirect-DMA gathers of v[src], v[dst] over the
     order-1 stream,
  2. dense per-edge current = |v_src - v_dst| * sigmoid(logit)/(R+X+eps),
  3. fp32 prefix scan of currents (global across partitions via a DRAM
     bounce), written doubled to DRAM,
  4. indirect-gather of the scan at node-boundary positions; adjacent
     differences = per-node segment sums (duplicate-safe, no scatters),
  5. order-2 currents via indirect permutation gather; same scan +
     boundary gather for the src sums,
  6. acc = dst_sums - src_sums all-reduced across cores; KVL variance
     partials all-reduced; final scalar loss on device.

TRN2 indirect-DMA hardware contract (probed on HW):
  - a 3-dim dest AP [[pstride,1],[8,N],[1,1]] based at partition p makes
    N one-element descriptors in partition p, 8 bytes apart;
  - source byte offset = idx * (dest run step) = idx*8 -> every
    gatherable DRAM array is stored as [M, 2] f32 rows (value at row
    start);
  - the offset AP is consumed as a flat stream in partition-fastest
    (column-major) order -> host packs each partition's index sequence
    into a [128, k] block whose column-major ravel is that sequence;
  - bounds_check skips OOB descriptors, leaving dest memory untouched.
The bass interpreter pairs offsets differently, so MultiCoreSim does NOT
validate the gathers; correctness is checked against the fp64 reference.
"""
import numpy as np

N_NODES = 100000
N_EDGES = 3200000
NCORES = 8
P = 128
EPC = N_EDGES // NCORES          # 400000 edges per core
C = EPC // P                     # 3125 edges per partition (partition-major)
NB = 782                         # node cols per partition; 128*782 >= N_NODES
GCOLS = NB + 1                   # boundary columns (one extra, shared edge)
BKV = (2 * C + P - 1) // P       # 49 offset-block cols for src+dst stream
BKP = (C + P - 1) // P           # 25 offset-block cols for pi stream
BKG = (2 * GCOLS + P - 1) // P   # 13 offset-block cols for boundary stream
SENT = 1 << 28                   # OOB sentinel for "position -1" boundaries
EPS = 1e-6

_cache = {}


def _build():
    import concourse.bass as bass
    import concourse.bacc as bacc
    import concourse.mybir as mybir
    from concourse.tile import TileContext

    f32 = mybir.dt.float32
    i32 = mybir.dt.int32

    nc = bacc.Bacc("TRN2", target_bir_lowering=False, debug=False, num_devices=NCORES)

    v_d = nc.dram_tensor("v", [N_NODES, 2], f32, kind="ExternalInput")
    gvv_d = nc.dram_tensor("gvv", [P, BKV * P], i32, kind="ExternalInput")
    gpi_d = nc.dram_tensor("gpi", [P, BKP * P], i32, kind="ExternalInput")
    gbb_d = nc.dram_tensor("gbb", [P, BKG * P], i32, kind="ExternalInput")
    lg_d = nc.dram_tensor("logits", [P, C], f32, kind="ExternalInput")
    r_d = nc.dram_tensor("rr", [P, C], f32, kind="ExternalInput")
    x_d = nc.dram_tensor("xx", [P, C], f32, kind="ExternalInput")
    out_d = nc.dram_tensor("out", [1, 1], f32, kind="ExternalOutput")

    # internal DRAM; gatherable arrays are [M, 2] f32 (8-byte rows).
    # pp_both holds both scans: order-1 at rows [0, P*C), order-2 above.
    cur_d = nc.dram_tensor("cur_dram", [P * C, 2], f32)
    pp_d = nc.dram_tensor("pp_dram", [2 * P * C, 2], f32)
    tot1_d = nc.dram_tensor("tot1", [P, 1], f32)
    row1_d = nc.dram_tensor("row1", [1, P], f32)
    tot2_d = nc.dram_tensor("tot2", [P, 1], f32)
    row2_d = nc.dram_tensor("row2", [1, P], f32)
    acc_d = nc.dram_tensor("acc_local", [P * NB, 1], f32)
    accr_d = nc.dram_tensor("acc_red", [P * NB, 1], f32)
    prt_d = nc.dram_tensor("prt_local", [1, 8], f32)
    prtr_d = nc.dram_tensor("prt_red", [1, 8], f32)

    cur_2d = cur_d[:, :].rearrange("(p c) two -> p (c two)", p=P)
    pp1_2d = pp_d[0:P * C, :].rearrange("(p c) two -> p (c two)", p=P)
    pp2_2d = pp_d[P * C:2 * P * C, :].rearrange("(p c) two -> p (c two)", p=P)
    acc_2d = acc_d[:, :].rearrange("(p c) o -> p (c o)", p=P)
    accr_2d = accr_d[:, :].rearrange("(p c) o -> p (c o)", p=P)

    AL = mybir.AluOpType
    AX = mybir.AxisListType
    AF = mybir.ActivationFunctionType

    with TileContext(nc) as tc:
        with (
            tc.tile_pool(name="big", bufs=1) as big,
            tc.tile_pool(name="sm", bufs=1) as sm,
            tc.tile_pool(name="ps", bufs=1, space="PSUM") as ps,
        ):
            # ---- load edge data ----
            gvv = big.tile([P, BKV * P], i32, tag="gvv")
            nc.sync.dma_start(out=gvv[:, :], in_=gvv_d[:, :])
            gpi = big.tile([P, BKP * P], i32, tag="gpi")
            nc.sync.dma_start(out=gpi[:, :], in_=gpi_d[:, :])
            gbb = sm.tile([P, BKG * P], i32, tag="gbb")
            nc.sync.dma_start(out=gbb[:, :], in_=gbb_d[:, :])
            lg = big.tile([P, C], f32, tag="wlg")
            nc.sync.dma_start(out=lg[:, :], in_=lg_d[:, :])
            rt = big.tile([P, C], f32, tag="rt")
            nc.sync.dma_start(out=rt[:, :], in_=r_d[:, :])
            xt = big.tile([P, C], f32, tag="sh3")
            nc.sync.dma_start(out=xt[:, :], in_=x_d[:, :])

            # ---- KVL partials: [sumR, sumR2, sumX, sumX2] ----
            ones = sm.tile([P, 1], f32, tag="ones")
            nc.vector.memset(ones[:, :], 1.0)
            red = sm.tile([P, 1], f32, tag="red")
            t1 = big.tile([P, C], f32, tag="t1")
            prt = sm.tile([1, 8], f32, tag="prt")
            nc.vector.memset(prt[:, :], 0.0)
            pssc = ps.tile([1, 1], f32, tag="pssc")
            for kk in range(4):
                colap = rt if kk < 2 else xt
                if kk % 2 == 0:
                    nc.vector.tensor_reduce(
                        out=red[:, :], in_=colap[:, :], axis=AX.X, op=AL.add
                    )
                else:
                    nc.vector.tensor_tensor(
                        out=t1[:, :], in0=colap[:, :], in1=colap[:, :], op=AL.mult
                    )
                    nc.vector.tensor_reduce(
                        out=red[:, :], in_=t1[:, :], axis=AX.X, op=AL.add
                    )
                nc.tensor.matmul(
                    pssc[:, :], lhsT=ones[:, :], rhs=red[:, :], start=True, stop=True
                )
                nc.vector.tensor_copy(prt[:, kk:kk + 1], pssc[:, :])
            nc.sync.dma_start(out=prt_d[:, :], in_=prt[:, :])

            # ---- dense edge weight w = sigmoid(logit) / (R + X + eps) ----
            nc.scalar.activation(t1[:, :], lg[:, :], AF.Sigmoid)
            nc.vector.tensor_tensor(out=xt[:, :], in0=rt[:, :], in1=xt[:, :], op=AL.add)
            nc.vector.tensor_scalar_add(xt[:, :], xt[:, :], EPS)
            w = lg  # reuse slot: logits are consumed
            nc.vector.reciprocal(w[:, :], xt[:, :])
            nc.vector.tensor_tensor(out=w[:, :], in0=w[:, :], in1=t1[:, :], op=AL.mult)

            # ---- per-partition gathers: v[src] then v[dst], 6250 descs each ----
            vvb = big.tile([P, 4 * C], f32, tag="vvb")
            vvb3 = vvb[:, :].rearrange("p (c two) -> p c two", two=2)
            for p in range(P):
                nc.gpsimd.indirect_dma_start(
                    out=vvb3[p:p + 1, :, 0:1], out_offset=None, in_=v_d[:, :],
                    in_offset=bass.IndirectOffsetOnAxis(
                        ap=gvv[:, BKV * p:BKV * (p + 1)], axis=0
                    ),
                )
            vsrc = vvb3[:, 0:C, 0]
            vdst = vvb3[:, C:2 * C, 0]

            # ---- per-edge current, doubled for the permutation gather ----
            curd = big.tile([P, 2 * C], f32, tag="share0")
            curd3 = curd[:, :].rearrange("p (c two) -> p c two", two=2)
            nc.vector.tensor_tensor(out=t1[:, :], in0=vsrc, in1=vdst, op=AL.subtract)
            nc.scalar.activation(curd3[:, :, 0], t1[:, :], AF.Abs)
            nc.vector.tensor_tensor(
                out=curd3[:, :, 0], in0=curd3[:, :, 0], in1=w[:, :], op=AL.mult
            )
            nc.vector.tensor_copy(curd3[:, :, 1], curd3[:, :, 0])
            nc.sync.dma_start(out=cur_2d, in_=curd[:, :])

            # ---- global prefix scan -> doubled DRAM ----
            tot = sm.tile([P, 1], f32, tag="tot")
            trow = sm.tile([1, P], f32, tag="trow")
            rrow = sm.tile([1, P], f32, tag="rrow")
            offs = sm.tile([P, 1], f32, tag="offs")

            def global_scan(src2d, tot_dram, row_dram, ppd, ppd3, pp_view):
                nc.vector.tensor_reduce(
                    out=tot[:, :], in_=src2d, axis=AX.X, op=AL.add
                )
                nc.sync.dma_start(out=tot_dram[:, :], in_=tot[:, :])
                nc.sync.dma_start(
                    out=trow[:, :], in_=tot_dram[:, :].rearrange("p o -> o p")
                )
                nc.vector.tensor_tensor_scan(
                    out=rrow[:, :], data0=trow[:, :], data1=trow[:, :],
                    initial=0.0, op0=AL.add, op1=AL.bypass,
                )
                nc.sync.dma_start(out=row_dram[:, :], in_=rrow[:, :])
                nc.vector.memset(offs[:, :], 0.0)
                nc.sync.dma_start(
                    out=offs[1:P, 0:1],
                    in_=row_dram[:, 0:P - 1].rearrange("o p -> p o"),
                )
                nc.vector.tensor_tensor_scan(
                    out=ppd3[:, :, 0], data0=src2d, data1=src2d,
                    initial=offs[:, 0:1], op0=AL.add, op1=AL.bypass,
                )
                nc.vector.tensor_copy(ppd3[:, :, 1], ppd3[:, :, 0])
                nc.sync.dma_start(out=pp_view, in_=ppd[:, :])

            ppd1 = big.tile([P, 2 * C], f32, tag="share1")
            ppd1_3 = ppd1[:, :].rearrange("p (c two) -> p c two", two=2)
            global_scan(curd3[:, :, 0], tot1_d, row1_d, ppd1, ppd1_3, pp1_2d)

            # ---- order-2: permutation gather of currents ----
            c2 = big.tile([P, 2 * C], f32, tag="share1")
            c23 = c2[:, :].rearrange("p (c two) -> p c two", two=2)
            for p in range(P):
                nc.gpsimd.indirect_dma_start(
                    out=c23[p:p + 1, :, 0:1], out_offset=None, in_=cur_d[:, :],
                    in_offset=bass.IndirectOffsetOnAxis(
                        ap=gpi[:, BKP * p:BKP * (p + 1)], axis=0
                    ),
                )

            ppd2 = big.tile([P, 2 * C], f32, tag="share0")
            ppd2_3 = ppd2[:, :].rearrange("p (c two) -> p c two", two=2)
            global_scan(c23[:, :, 0], tot2_d, row2_d, ppd2, ppd2_3, pp2_2d)

            # ---- boundary gathers (both orders in one stream) ----
            gg = big.tile([P, 4 * GCOLS], f32, tag="rt")
            gg3 = gg[:, :].rearrange("p (c two) -> p c two", two=2)
            nc.vector.memset(gg[:, :], 0.0)
            for p in range(P):
                nc.gpsimd.indirect_dma_start(
                    out=gg3[p:p + 1, :, 0:1], out_offset=None, in_=pp_d[:, :],
                    in_offset=bass.IndirectOffsetOnAxis(
                        ap=gbb[:, BKG * p:BKG * (p + 1)], axis=0
                    ),
                    bounds_check=2 * P * C - 1, oob_is_err=False,
                )
            acc = sm.tile([P, NB], f32, tag="acc")
            nc.vector.tensor_tensor(
                out=acc[:, :], in0=gg3[:, 1:GCOLS, 0], in1=gg3[:, 0:NB, 0],
                op=AL.subtract,
            )
            tmp = sm.tile([P, NB], f32, tag="tmp")
            nc.vector.tensor_tensor(
                out=tmp[:, :], in0=gg3[:, GCOLS + 1:2 * GCOLS, 0],
                in1=gg3[:, GCOLS:GCOLS + NB, 0], op=AL.subtract,
            )
            nc.vector.tensor_tensor(
                out=acc[:, :], in0=acc[:, :], in1=tmp[:, :], op=AL.subtract
            )
            nc.sync.dma_start(out=acc_2d, in_=acc[:, :])

            # ---- all-reduce across the 8 cores ----
            nc.gpsimd.collective_compute(
                "AllReduce",
                AL.add,
                replica_groups=[list(range(NCORES))],
                ins=[acc_d.ap().opt()],
                outs=[accr_d.ap().opt()],
            )
            nc.gpsimd.collective_compute(
                "AllReduce",
                AL.add,
                replica_groups=[list(range(NCORES))],
                ins=[prt_d.ap().opt()],
                outs=[prtr_d.ap().opt()],
            )

            # ---- final loss ----
            nst = big.tile([P, NB], f32, tag="sh3")
            nc.sync.dma_start(out=nst[:, :], in_=accr_2d)
            nc.vector.tensor_tensor(
                out=tmp[:, :], in0=nst[:, :], in1=nst[:, :], op=AL.mult
            )
            nc.vector.tensor_reduce(
                out=red[:, :], in_=tmp[:, :], axis=AX.X, op=AL.add
            )
            kclp = ps.tile([1, 1], f32, tag="kclp")
            nc.tensor.matmul(
                kclp[:, :], lhsT=ones[:, :], rhs=red[:, :], start=True, stop=True
            )
            kcl = sm.tile([1, 1], f32, tag="kcl")
            nc.vector.tensor_scalar_mul(kcl[:, :], kclp[:, :], 1.0 / N_NODES)

            prtf = sm.tile([1, 8], f32, tag="prtf")
            nc.sync.dma_start(out=prtf[:, :], in_=prtr_d[:, :])
            E = float(N_EDGES)
            meanterm = sm.tile([1, 2], f32, tag="meanterm")
            s1 = prtf[:, :].rearrange("o (a b) -> o a b", b=2)[:, 0:2, 0]
            s2 = prtf[:, :].rearrange("o (a b) -> o a b", b=2)[:, 0:2, 1]
            nc.vector.tensor_tensor(
                out=meanterm[:, :], in0=s1, in1=s1, op=AL.mult
            )
            nc.vector.tensor_scalar_mul(meanterm[:, :], meanterm[:, :], -1.0 / E)
            nc.vector.tensor_tensor(
                out=meanterm[:, :], in0=meanterm[:, :], in1=s2, op=AL.add
            )
            kvl = sm.tile([1, 1], f32, tag="kvl")
            nc.vector.tensor_reduce(
                out=kvl[:, :], in_=meanterm[:, :], axis=AX.X, op=AL.add
            )
            nc.vector.tensor_scalar_mul(kvl[:, :], kvl[:, :], 0.5 / (E - 1.0))

            res = sm.tile([1, 1], f32, tag="res")
            nc.vector.tensor_tensor(
                out=res[:, :], in0=kcl[:, :], in1=kvl[:, :], op=AL.add
            )
            nc.sync.dma_start(out=out_d[:, :], in_=res[:, :])

    nc.compile()
    return nc


def _pack_blocks(seqs, bk):
    """Pack per-partition index sequences into a [P, bk*P] offset tile whose
    per-partition [P, bk] block has the sequence as its column-major ravel."""
    out = np.zeros((P, bk * P), np.int32)
    for p in range(P):
        stream = np.zeros(bk * P, np.int64)
        stream[:len(seqs[p])] = seqs[p]
        out[:, bk * p:bk * (p + 1)] = stream.reshape(bk, P).T.astype(np.int32)
    return out


def _prepare_in_maps(node_features, edge_index, edge_logits, edge_params):
    vcol = np.asarray(node_features[:, 0], dtype=np.float32)
    v2 = np.stack([vcol, vcol], axis=1)  # 8-byte rows for the gather ucode
    src = np.asarray(edge_index[0], dtype=np.int64)
    dst = np.asarray(edge_index[1], dtype=np.int64)
    lg = np.asarray(edge_logits, dtype=np.float32)
    R = np.ascontiguousarray(edge_params[:, 0], dtype=np.float32)
    X = np.ascontiguousarray(edge_params[:, 1], dtype=np.float32)

    node_ids = (
        np.arange(P, dtype=np.int64)[:, None] * NB
        + np.arange(GCOLS, dtype=np.int64)[None, :]
    )  # [128, 783]; last col == next partition's first node

    in_maps = []
    for k in range(NCORES):
        sl = slice(k * EPC, (k + 1) * EPC)
        s_, d_ = src[sl], dst[sl]
        o1 = np.argsort(d_, kind="stable")
        o2 = np.argsort(s_, kind="stable")
        d1 = d_[o1]
        s2s = s_[o2]

        S1 = np.searchsorted(d1, node_ids.ravel(), side="left").reshape(P, GCOLS)
        b1 = S1.astype(np.int64) - 1
        b1[S1 == 0] = SENT
        S2 = np.searchsorted(s2s, node_ids.ravel(), side="left").reshape(P, GCOLS)
        b2 = S2.astype(np.int64) - 1 + P * C
        b2[S2 == 0] = SENT

        pos1 = np.empty(EPC, dtype=np.int64)
        pos1[o1] = np.arange(EPC, dtype=np.int64)
        pi = pos1[o2].reshape(P, C)

        s1r = s_[o1].reshape(P, C)
        d1r = d1.reshape(P, C)

        in_maps.append({
            "v": v2,
            "gvv": _pack_blocks(
                [np.concatenate([s1r[p], d1r[p]]) for p in range(P)], BKV
            ),
            "gpi": _pack_blocks([pi[p] for p in range(P)], BKP),
            "gbb": _pack_blocks(
                [np.concatenate([b1[p], b2[p]]) for p in range(P)], BKG
            ),
            "logits": np.ascontiguousarray(lg[sl][o1].reshape(P, C)),
            "rr": np.ascontiguousarray(R[sl][o1].reshape(P, C)),
            "xx": np.ascontiguousarray(X[sl][o1].reshape(P, C)),
        })
    return in_maps


def kernel(node_features, edge_index, edge_logits, edge_params):
    from concourse.bass_utils import run_bass_kernel_spmd

    if "nc" not in _cache:
        _cache["nc"] = _build()
    nc = _cache["nc"]

    in_maps = _prepare_in_maps(node_features, edge_index, edge_logits, edge_params)
    globals()["_last_in_maps"] = in_maps

    res = run_bass_kernel_spmd(nc, in_maps, core_ids=list(range(NCORES)))
    return np.float32(res.results[0]["out"][0, 0])
